# revision 40
# baseline (speedup 1.0000x reference)
"""Fused multi-head attention (B=2, T=2048, D=2048, H=16) on 8 trn2 NeuronCores.

Sharding: core c handles batch b=c//4 and heads [4g, 4g+4), g=c%4 (tensor
parallel over heads x data parallel over batch). Each core computes its
4 heads' contribution to out[b] = attn(x[b]) @ Wo^T; the host sums the 4
bf16 partials per batch in f32.

All matmul operands are bf16 (PSUM accumulates f32): same 1-cycle/row PE
rate as f32r but half the LDWEIGHTS/DMA/SBUF cost (rel err ~6e-3). The
host packs every DRAM tensor so each DMA line is multi-KB contiguous, and
the issue order puts first-needed tiles (wq/wk m=0, x dt=0..3) at the
front of the queue FIFOs so the PE starts ~10us into the kernel.

Device algorithm (x fully SBUF-resident, loaded once):
  P1qk fused per m-block: qT/kT[e,t] += Wq/Wk-block^T @ x-chunk
       (8 psum banks, m=0 chases the x DMA stream)
  P1v  v[t,e] = x-block^T @ full-width Wv rows -> no transposes; reuses the
       qk psum names (alternating by chunk parity) so no pool boundary
  P2   per i-chunk (512 queries), per head, per surviving key block jt:
         S^T[j,i] (PE) -> exp (ACT, bf16) -> *mask-block (DVE, mixed only)
         ctx^T[e,i] += v_h^T @ P^T ; all-ones[128,128] stationary gives
         l[i] pre-broadcast on all partitions (no separate bcast matmul)
       diagonal blocks trimmed to their unmasked column range;
       epilogue: reciprocal_approx_fast [128,512] -> ctx *= 1/l (DVE)
  P3   out[t,:] = sum_h ctx_h^T-block @ Wo rows -> bf16 staging -> one
       row-block DMA per token tile. One tile of chunk ic-1 is interleaved
       after each P2 head of chunk ic (pure-PE filler for exp-wait
       hiccups, 2-bank psum ping-pong); only the last chunk's 4 tiles run
       as a tail.
"""

import numpy as np
import ml_dtypes

import concourse.bass as bass
import concourse.mybir as mybir
import concourse.tile as tile
from concourse import bacc
from concourse.bass_utils import run_bass_kernel_spmd

F32 = mybir.dt.float32
BF16 = mybir.dt.bfloat16
EXP = mybir.ActivationFunctionType.Exp
BF = ml_dtypes.bfloat16

B, T, D, H = 2, 2048, 2048, 16
DH = D // H          # 128
E = 512              # features per core (4 heads)
HPC = 4              # heads per core
NT = T // 128        # 16 token tiles
ND = D // 128        # 16 model-dim tiles
NE = E // 128        # 4 e-tiles per core
NI = T // 512        # 4 i-chunks
NJ = NT              # 16 j-tiles

_NC_CACHE = {}

SKIP, NOMULT, MIXED = 0, 1, 2


def _build(key):
    cls_key, trim_key = key
    cls = np.asarray(cls_key, dtype=np.int64).reshape(NJ, NI)
    trim = {(jt, ic): (c0, c1) for jt, ic, c0, c1 in trim_key}
    # mixed-block table: order of blocks inside the packed `emp` tensor
    mixidx = {}
    nmix = 0
    for ic in range(NI):
        for jt in range(NJ):
            if cls[jt, ic] == MIXED:
                mixidx[(jt, ic)] = nmix
                nmix += 1

    nc = bacc.Bacc(None, target_bir_lowering=False, debug=False)
    xp = nc.declare_dram_parameter("xp", [128, ND * T], BF16, isOutput=False)
    wqp = nc.declare_dram_parameter("wqp", [128, NE * ND * 128], BF16,
                                    isOutput=False)
    wkp = nc.declare_dram_parameter("wkp", [128, NE * ND * 128], BF16,
                                    isOutput=False)
    wvp = nc.declare_dram_parameter("wvp", [128, ND * E], BF16, isOutput=False)
    wop = nc.declare_dram_parameter("wop", [128, HPC * D], BF16, isOutput=False)
    emp = nc.declare_dram_parameter("emp", [128, max(nmix, 1) * 512], BF16,
                                    isOutput=False)
    onb = nc.declare_dram_parameter("onb", [128, 128], BF16, isOutput=False)
    out = nc.declare_dram_parameter("out", [T, D], BF16, isOutput=True)

    with tile.TileContext(nc) as tc:
        # ---- long-lived residents ---------------------------------------
        p_const = tc.alloc_tile_pool(name="consts", bufs=1)
        ones_b = p_const.tile([128, 128], BF16)
        nc.sync.dma_start(out=ones_b, in_=onb.ap())

        # em/wo live here (allocated early) so their DMAs issue at t=0
        # instead of behind the P1 pool-release boundary
        pool_res0 = tc.alloc_tile_pool(name="res_emwo", bufs=1)
        wo_sb = pool_res0.tile([128, HPC, D], BF16, name="wo_sb")
        em_sb = pool_res0.tile([128, max(nmix, 1), 512], BF16, name="em_sb")

        pool_res1 = tc.alloc_tile_pool(name="res_qkv", bufs=1)
        qT = pool_res1.tile([128, HPC, T], BF16, name="qT")
        kT = pool_res1.tile([128, HPC, T], BF16, name="kT")
        v_sb = pool_res1.tile([128, NT, E], BF16, name="v_sb")

        # ---- P1: projections --------------------------------------------
        scope_p1 = nc.named_scope("P1_qkv"); scope_p1.__enter__()
        p_w = tc.alloc_tile_pool(name="p1w", bufs=1)
        wq_sb = p_w.tile([128, NE, ND, 128], BF16, name="wq_sb")
        wk_sb = p_w.tile([128, NE, ND, 128], BF16, name="wk_sb")
        wv_sb = p_w.tile([128, ND, E], BF16, name="wv_sb")
        p_x = tc.alloc_tile_pool(name="p1x", bufs=1)
        x_sb = p_x.tile([128, ND, T], BF16, name="x_sb")
        # DMA order = queue FIFO order: first-needed m0 weight quarters and x
        # tiles interleaved so the PE can start after ~0.75MB instead of 1.5MB
        W128 = ND * 128
        qtr = W128 // 4
        for q in range(4):
            nc.sync.dma_start(out=wq_sb[:, 0, 4 * q:4 * q + 4, :],
                              in_=wqp.ap()[:, q * qtr:(q + 1) * qtr])
            nc.sync.dma_start(out=wk_sb[:, 0, 4 * q:4 * q + 4, :],
                              in_=wkp.ap()[:, q * qtr:(q + 1) * qtr])
            nc.sync.dma_start(out=x_sb[:, q, :],
                              in_=xp.ap()[:, q * T:(q + 1) * T])
        for dt in range(4, ND):
            nc.sync.dma_start(out=x_sb[:, dt, :], in_=xp.ap()[:, dt * T:(dt + 1) * T])
        for m in range(1, NE):
            nc.sync.dma_start(out=wq_sb[:, m, :, :],
                              in_=wqp.ap()[:, m * W128:(m + 1) * W128])
            nc.sync.dma_start(out=wk_sb[:, m, :, :],
                              in_=wkp.ap()[:, m * W128:(m + 1) * W128])
        nc.sync.dma_start(out=wv_sb, in_=wvp.ap())
        nc.sync.dma_start(out=em_sb, in_=emp.ap())
        nc.sync.dma_start(out=wo_sb, in_=wop.ap())

        p_ps1 = tc.alloc_tile_pool(name="p1ps", bufs=8, space="PSUM")
        # fused q+k pass per m (8 psum banks; x DMA chased during m=0)
        for m in range(NE):
            psq = [p_ps1.tile([128, 512], F32, name=f"psq{i}", bufs=1)
                   for i in range(NI)]
            psk = [p_ps1.tile([128, 512], F32, name=f"psk{i}", bufs=1)
                   for i in range(NI)]
            for dt in range(ND):
                st, sp = dt == 0, dt == ND - 1
                for i in range(NI):
                    nc.tensor.matmul(psq[i], wq_sb[:, m, dt, :],
                                     x_sb[:, dt, i * 512:(i + 1) * 512],
                                     start=st, stop=sp)
                    nc.tensor.matmul(psk[i], wk_sb[:, m, dt, :],
                                     x_sb[:, dt, i * 512:(i + 1) * 512],
                                     start=st, stop=sp)
            # drain copies in the order the next m's matmuls reclaim the
            # banks (q0,k0,q1,k1,...), split across ACT and DVE so the
            # m-boundary stall is one copy deep, not four
            for i in range(NI):
                if i % 2 == 0:
                    nc.scalar.copy(qT[:, m, i * 512:(i + 1) * 512], psq[i])
                    nc.vector.tensor_copy(kT[:, m, i * 512:(i + 1) * 512], psk[i])
                else:
                    nc.vector.tensor_copy(qT[:, m, i * 512:(i + 1) * 512], psq[i])
                    nc.scalar.copy(kT[:, m, i * 512:(i + 1) * 512], psk[i])
        # v pass: stationary = x block, moving = full-width wv row.
        # Reuses the qk psum names (alternating by chunk parity) so there is
        # no pool boundary and chunks stay double-buffered.
        for nch in range(NI):
            grp = "psq" if nch % 2 == 0 else "psk"
            ps = [p_ps1.tile([128, 512], F32, name=f"{grp}{i}", bufs=1)
                  for i in range(NI)]
            for dt in range(ND):
                st, sp = dt == 0, dt == ND - 1
                for tl in range(4):
                    base = nch * 512 + tl * 128
                    nc.tensor.matmul(ps[tl], x_sb[:, dt, base:base + 128],
                                     wv_sb[:, dt, :], start=st, stop=sp)
            for tl in range(4):
                eng = nc.vector.tensor_copy if tl % 2 else nc.scalar.copy
                eng(v_sb[:, nch * 4 + tl, :], ps[tl])
        p_ps1.release()
        p_x.release()
        p_w.release()
        scope_p1.__exit__(None, None, None)

        # ---- P2: attention ----------------------------------------------
        scope_p2 = nc.named_scope("P2_attn"); scope_p2.__enter__()
        pool_res2 = tc.alloc_tile_pool(name="res2", bufs=1)
        ctx = pool_res2.tile([128, HPC, T], BF16, name="ctx")

        p_pt = tc.alloc_tile_pool(name="p2pt", bufs=3)
        p_ptm = tc.alloc_tile_pool(name="p2ptm", bufs=2)
        p_rr = tc.alloc_tile_pool(name="p2rr", bufs=2)
        ps_s_pool = tc.alloc_tile_pool(name="p2pss", bufs=3, space="PSUM")
        ps_ctx_pool = tc.alloc_tile_pool(name="p2psc", bufs=2, space="PSUM")
        ps_l_pool = tc.alloc_tile_pool(name="p2psl", bufs=1, space="PSUM")
        # 2-bank ping-pong for the output projection tiles interleaved
        # between P2 heads (their matmuls have no ACT dependency, so they
        # fill the PE's exp-wait hiccups; copies go to DVE since ACT is
        # the scarce engine here)
        ps_o_pool = tc.alloc_tile_pool(name="p3ps", bufs=2, space="PSUM")
        p_stg = tc.alloc_tile_pool(name="p3stg", bufs=2)

        def p3_tile(tt):
            # generator: each next() emits ONE projection matmul so the
            # caller can drip them between attention blocks as pure-PE
            # filler for exp-wait stalls
            tsl = slice(tt * 128, (tt + 1) * 128)
            stg = p_stg.tile([128, T], BF16, name="stg", bufs=2)
            for nch in range(NI):
                ps_o = ps_o_pool.tile([128, 512], F32, name=f"pso{nch % 2}",
                                      bufs=1)
                for h in range(HPC):
                    nc.tensor.matmul(
                        ps_o, ctx[:, h, tsl],
                        wo_sb[:, h, nch * 512:(nch + 1) * 512],
                        start=h == 0, stop=h == HPC - 1)
                    yield
                eng = nc.vector.tensor_copy if nch % 2 else nc.scalar.copy
                eng(stg[:, nch * 512:(nch + 1) * 512], ps_o)
            if tt == NT - 1:
                # split the final DMA so the kernel end isn't serialized
                # behind one full-row transfer
                nc.sync.dma_start(out=out.ap()[tsl, :1024], in_=stg[:, :1024])
                nc.sync.dma_start(out=out.ap()[tsl, 1024:], in_=stg[:, 1024:])
            else:
                nc.sync.dma_start(out=out.ap()[tsl, :], in_=stg)

        for ic in range(NI):
            isl = slice(ic * 512, (ic + 1) * 512)
            surv = [jt for jt in range(NJ) if cls[jt, ic] != SKIP]
            assert surv, f"i-chunk {ic}: every key block masked"
            first, last = surv[0], surv[-1]
            for h in range(HPC):
                fill = p3_tile(4 * (ic - 1) + h) if ic > 0 else iter(())
                cps = ps_ctx_pool.tile([128, 512], F32, name="ps_c", bufs=2)
                # all-ones stationary -> l arrives broadcast on all partitions
                lps = ps_l_pool.tile([128, 512], F32, name="ps_l", bufs=1)
                for jt in surv:
                    # column range with any unmasked key (trim masked-out
                    # moving columns; first block stays full width so the
                    # start=True zero-fill covers the whole accumulator)
                    c0, c1 = trim.get((jt, ic), (0, 512))
                    if jt == first:
                        c0, c1 = 0, 512
                    csl = slice(ic * 512 + c0, ic * 512 + c1)
                    ps_s = ps_s_pool.tile([128, 512], F32, name="ps_s", bufs=3)
                    nc.tensor.matmul(
                        ps_s[:, c0:c1], kT[:, h, jt * 128:(jt + 1) * 128],
                        qT[:, h, csl], start=True, stop=True)
                    pt = p_pt.tile([128, 512], BF16, name="pt", bufs=4)
                    nc.scalar.activation(pt[:, c0:c1], ps_s[:, c0:c1], EXP)
                    if cls[jt, ic] == MIXED:
                        ptm = p_ptm.tile([128, 512], BF16, name="ptm", bufs=2)
                        nc.vector.tensor_mul(
                            ptm[:, c0:c1], pt[:, c0:c1],
                            em_sb[:, mixidx[(jt, ic)], c0:c1])
                    else:
                        ptm = pt
                    st, sp = jt == first, jt == last
                    nc.tensor.matmul(
                        cps[:, c0:c1], v_sb[:, jt, h * 128:(h + 1) * 128],
                        ptm[:, c0:c1], start=st, stop=sp)
                    nc.tensor.matmul(lps[:, c0:c1], ones_b, ptm[:, c0:c1],
                                     start=st, stop=sp)
                    next(fill, None)
                rr = p_rr.tile([128, 512], F32, name="rr", bufs=2)
                nc.vector.reciprocal_approx_fast(out=rr, in_=lps)
                nc.vector.tensor_mul(ctx[:, h, isl], cps, rr)
                for _ in fill:
                    pass
        scope_p2.__exit__(None, None, None)

        # ---- P3 tail: last chunk's output tiles -------------------------
        scope_p3 = nc.named_scope("P3_out"); scope_p3.__enter__()
        for tt in range(4 * (NI - 1), NT):
            for _ in p3_tile(tt):
                pass
        for p in (p_stg, ps_o_pool, ps_l_pool, ps_ctx_pool, ps_s_pool,
                  p_rr, p_ptm, p_pt):
            p.release()
        pool_res2.release()
        pool_res1.release()
        pool_res0.release()
        p_const.release()
        scope_p3.__exit__(None, None, None)

    nc.compile()
    return nc


def _get_nc(key):
    if key not in _NC_CACHE:
        _NC_CACHE[key] = _build(key)
    return _NC_CACHE[key]


def _pack_rows(a):
    """[D, N] f32 -> [128, (D//128)*N] bf16 with d-tiles side by side."""
    d, n = a.shape
    return np.ascontiguousarray(
        a.reshape(d // 128, 128, n).transpose(1, 0, 2).reshape(128, -1)
    ).astype(BF)


def _pack_rows_m(a):
    """[D, E] f32 -> [128, NE*ND*128] bf16, m-major (e-block outer)."""
    d, e = a.shape
    return np.ascontiguousarray(
        a.reshape(d // 128, 128, e // 128, 128).transpose(1, 2, 0, 3)
        .reshape(128, -1)
    ).astype(BF)


def kernel(x, Wq, Wk, Wv, Wo, attn_mask):
    x = np.asarray(x, dtype=np.float32)
    Wq = np.asarray(Wq, dtype=np.float32)
    Wk = np.asarray(Wk, dtype=np.float32)
    Wv = np.asarray(Wv, dtype=np.float32)
    Wo = np.asarray(Wo, dtype=np.float32)
    mask = np.asarray(attn_mask, dtype=np.float32).reshape(T, T)

    emT = np.ascontiguousarray(np.exp(mask).T)          # [j, i]
    scale = np.float32(1.0 / np.sqrt(DH))

    blocks = emT.reshape(NJ, 128, NI, 512)
    cls = np.full((NJ, NI), MIXED, dtype=np.int64)
    for jt in range(NJ):
        for ic in range(NI):
            sub = blocks[jt, :, ic, :]
            if not sub.any():
                cls[jt, ic] = SKIP
            elif np.all(sub == 1.0):
                cls[jt, ic] = NOMULT
    cls_key = tuple(cls.flatten().tolist())

    # per-mixed-block unmasked column range (for moving-dim trimming)
    trims = []
    for ic in range(NI):
        for jt in range(NJ):
            if cls[jt, ic] == MIXED:
                cols = np.nonzero(blocks[jt, :, ic, :].any(axis=0))[0]
                trims.append((jt, ic, int(cols.min()), int(cols.max()) + 1))
    key = (cls_key, tuple(trims))

    # packed mixed blocks, ordered by (ic, jt)
    mix = [emT[jt * 128:(jt + 1) * 128, ic * 512:(ic + 1) * 512]
           for ic in range(NI) for jt in range(NJ) if cls[jt, ic] == MIXED]
    if mix:
        emp = np.ascontiguousarray(
            np.stack(mix, axis=1).reshape(128, -1)).astype(BF)
    else:
        emp = np.zeros((128, 512), dtype=BF)

    in_maps = []
    for c in range(8):
        b, g = c // 4, c % 4
        rows = slice(E * g, E * (g + 1))
        in_maps.append({
            "xp": _pack_rows(np.ascontiguousarray(x[b].T)),
            "wqp": _pack_rows_m(np.ascontiguousarray((Wq[rows, :] * scale).T)),
            "wkp": _pack_rows_m(np.ascontiguousarray(Wk[rows, :].T)),
            "wvp": _pack_rows(np.ascontiguousarray(Wv[rows, :].T)),
            "wop": _pack_rows(np.ascontiguousarray(Wo[:, rows].T)),
            "emp": emp,
            "onb": np.ones((128, 128), dtype=BF),
        })

    global _LAST_IN_MAPS, _LAST_NC
    _LAST_IN_MAPS = in_maps
    nc = _get_nc(key)
    _LAST_NC = nc
    res = run_bass_kernel_spmd(nc, in_maps, list(range(8)))
    outs = [np.asarray(r["out"]).astype(np.float32) for r in res.results]
    full = np.stack([
        outs[0] + outs[1] + outs[2] + outs[3],
        outs[4] + outs[5] + outs[6] + outs[7],
    ]).astype(np.float32)
    return full
